# revision 25
# baseline (speedup 1.0000x reference)
"""Trainium2 Bass kernel for nn_Attention_54254026883778.

Single-head attention with an additive post-softmax intensity term:
    q/k/v = X @ W{q,k,v}.T + b;  scores = q k^T / sqrt(D)
    attn  = softmax(scores) + intensity;  out = (attn @ v) @ Wo.T + bo

Sharding: 8 cores = 4 batches x 2 sequence halves. Each core computes
K^T and V for its whole batch (duplicated across the pair) and Q/attention
for its own 1024 query rows. No collectives. The host rolls the t-axis per
core so the core's own query rows are always at t-positions 0..SH; K/V and
intensity follow the same rolled order, which leaves attn @ V invariant.

Device dataflow (host pre-transposes X, W and intensity so contraction /
partition dims land where the engines want them):
    Q^T/K^T  [dout | s]  = WxT-chunk.T @ XT        (lhsT=WxT, rhs=XT)
    V        [t | dv]    = XT-chunk.T @ WvT        (lhsT=XT,  rhs=WvT)
    scores   [s | t]     = QT-chunk.T @ KT  -> exp on ACT (no max-subtract:
        |scores| < ~3) with fused row-accumulate -> 1/den on DVE ->
        diag(recip) = ident * recip (DVE) ->
        attn^T tile = E-slice.T @ diag(recip)      (one PE matmul both
        transposes and normalizes) -> DVE copy adds intensity^T (bf16,
        host-transposed) while draining PSUM -> attn^T [t | s]
    out^T    [dv | s]    = V-chunk.T @ attn^T
    final^T  [do | s]    = WoT-chunk.T @ out^T     -> DRAM, host transposes
Biases enter as rank-1 fp32r matmuls accumulated into PSUM:
    q/k/o:  b_row (x) ones_row;   v: bv_row (x) attn-rowsums, where the
    rowsums are 1 + rowsum(intensity) (softmax rows sum to 1), shipped
    from the host inside the bias pack.
"""

import numpy as np
import ml_dtypes

P = 128
D = 1024
S = 2048          # keys per batch (full sequence)
SH = 1024         # query rows owned by each core
DC = D // P       # 8  contraction chunks over model dim
TC = S // P       # 16 t (key) chunks
NT = 512          # matmul moving free dim / psum bank
SJ = SH // NT     # 2  s-tiles of own rows
TJ = S // NT      # 4  t-tiles
SCALE = 1.0 / 32.0  # 1/sqrt(D)

_CACHE = {}


def _build_module():
    import concourse.bass as bass
    import concourse.tile as tile
    import concourse.mybir as mybir
    from concourse import bacc
    from concourse.masks import make_identity

    f32 = mybir.dt.float32
    f32r = mybir.dt.float32r
    bf16 = mybir.dt.bfloat16
    Exp = mybir.ActivationFunctionType.Exp
    add = mybir.AluOpType.add

    nc = bacc.Bacc("TRN2", target_bir_lowering=False, debug=False,
                   num_devices=8)

    XQ_d = nc.dram_tensor("XQT", [D, SH], bf16, kind="ExternalInput")
    WQ_d = nc.dram_tensor("WQT", [D, D], bf16, kind="ExternalInput")
    WK_d = nc.dram_tensor("WKT", [D, D], bf16, kind="ExternalInput")
    WV_d = nc.dram_tensor("WVT", [D, D], bf16, kind="ExternalInput")
    WO_d = nc.dram_tensor("WOT", [D, D], bf16, kind="ExternalInput")
    BCOL_d = nc.dram_tensor("BCOL", [P, 3 * DC], f32, kind="ExternalInput")
    BROW_d = nc.dram_tensor("BROW", [1, D + SH], f32, kind="ExternalInput")
    IT_d = nc.dram_tensor("IT", [SH, S], bf16, kind="ExternalInput")
    OUT_d = nc.dram_tensor("OUTT", [D, SH], f32, kind="ExternalOutput")

    xq_v = XQ_d[:].rearrange("(c p) s -> p c s", p=P)
    wq_v = WQ_d[:].rearrange("(c p) o -> p c o", p=P)
    wk_v = WK_d[:].rearrange("(c p) o -> p c o", p=P)
    wv_v = WV_d[:].rearrange("(c p) o -> p c o", p=P)
    wo_v = WO_d[:].rearrange("(c p) o -> p c o", p=P)
    it_v = IT_d[:].rearrange("(si p) f -> si p f", p=P)  # per-si blocks
    out_v = OUT_d[:].rearrange("(c p) s -> p c s", p=P)

    with tile.TileContext(nc) as tc:
        with (
            tc.tile_pool(name="persist", bufs=1) as persist,
            tc.tile_pool(name="mm_ps", bufs=4, space="PSUM") as mm_ps,
            tc.tile_pool(name="tr_ps", bufs=4, space="PSUM") as tr_ps,
            tc.tile_pool(name="dram", bufs=1, space="DRAM") as dram_pool,
        ):
            GROUPS = [[0, 1], [2, 3], [4, 5], [6, 7]]
            # ---- persistent tiles -------------------------------------
            KT_sb = persist.tile([P, 2, SJ, DC, NT], bf16)  # K^T [d | blk,tt]
            V_sb = persist.tile([P, TC, D], bf16)           # V natural [t | dv]
            QT_sb = persist.tile([P, DC, SH], bf16, tag="qt_ot")   # Q^T [d | s]

            ident = persist.tile([P, P], bf16)
            make_identity(nc, ident)
            # bq|bk|bo as per-partition columns, added during PSUM extract
            BCOL_sb = persist.tile([P, 3 * DC], f32)
            nc.sync.dma_start(BCOL_sb[:], BCOL_d[:])
            # bv and the attn rowsums feed the PV rank-1 bias matmul; fp32r
            # operands must come from a rounding instruction, so stage the
            # DMA through a DVE copy.
            BROW_ld = persist.tile([1, D + SH], f32)
            nc.sync.dma_start(BROW_ld[:], BROW_d[:])
            BROW_r = persist.tile([1, D + SH], f32r)
            nc.vector.tensor_copy(out=BROW_r[:], in_=BROW_ld[:])
            BV_sb = BROW_r[0:1, 0:D]
            RS_sb = BROW_r[0:1, D:D + SH]                   # 1 + rowsum(I)

            # ---- phase A: QKV (local rows) + pair AllGather of K/V ----
            # Each core computes Q^T, and K^T/V for ITS OWN 1024 rows only
            # (from XQT); the partner's half arrives via a 2-rank AllGather
            # through shared DRAM, placed in global t-order (SPMD-static).
            XQ_sb = persist.tile([P, DC, SH], bf16, tag="xq_at")
            KL_sb = persist.tile([P, SJ, DC, NT], bf16)  # local K^T [tt | d]
            VL_sb = persist.tile([P, DC, SH], bf16)      # local V [t-own | dv]
            with tc.tile_pool(name="wpool", bufs=2) as wpool:
                WK_sb = wpool.tile([P, DC, D], bf16, tag="w")
                WQ_sb = wpool.tile([P, DC, D], bf16, tag="w")
                # Per-chunk DMAs, K operands first: K-local runs first so its
                # AllGather kicks off as early as possible and hides under
                # the Q and V-local compute.
                for dc in range(DC):
                    nc.sync.dma_start(XQ_sb[:, dc, :], xq_v[:, dc, :])
                    nc.sync.dma_start(WK_sb[:, dc, :], wk_v[:, dc, :])
                for dc in range(DC):
                    nc.sync.dma_start(WQ_sb[:, dc, :], wq_v[:, dc, :])

                # K^T local [dout, t-own], t-tile outer so each half's
                # AllGather kicks off as soon as that half is computed
                for j in range(SJ):
                    for c in range(DC):
                        ps = mm_ps.tile([P, NT], f32, tag="mm", name="ps")
                        for dc in range(DC):
                            nc.tensor.matmul(
                                ps[:],
                                WK_sb[:, dc, c * P:(c + 1) * P],
                                XQ_sb[:, dc, j * NT:(j + 1) * NT],
                                start=(dc == 0), stop=(dc == DC - 1),
                            )
                        nc.vector.tensor_scalar_add(
                            KL_sb[:, j, c, :], ps[:],
                            BCOL_sb[:, DC + c:DC + c + 1])
                    k_in = dram_pool.tile([P, DC, NT], bf16, name="k_in")
                    k_out = dram_pool.tile([2, P, DC, NT], bf16, name="k_out")
                    nc.sync.dma_start(k_in[:], KL_sb[:, j])
                    nc.gpsimd.collective_compute(
                        "AllGather", mybir.AluOpType.bypass,
                        replica_groups=GROUPS,
                        ins=[k_in.opt()], outs=[k_out.opt()])
                    nc.sync.dma_start(KT_sb[:, 0, j], k_out[0])
                    nc.sync.dma_start(KT_sb[:, 1, j], k_out[1])

                # Q^T [dout, s-own]
                for c in range(DC):
                    psl = [mm_ps.tile([P, NT], f32, tag="mm", name="ps")
                           for _ in range(SJ)]
                    for dc in range(DC):
                        for j in range(SJ):
                            nc.tensor.matmul(
                                psl[j][:],
                                WQ_sb[:, dc, c * P:(c + 1) * P],
                                XQ_sb[:, dc, j * NT:(j + 1) * NT],
                                start=(dc == 0), stop=(dc == DC - 1),
                            )
                    for j in range(SJ):
                        nc.vector.tensor_scalar_add(
                            QT_sb[:, c, j * NT:(j + 1) * NT], psl[j][:],
                            BCOL_sb[:, c:c + 1])

                WV_sb = wpool.tile([P, DC, D], bf16, tag="w")
                for dc in range(DC):
                    nc.sync.dma_start(WV_sb[:, dc, :], wv_v[:, dc, :])
                # V local [t-own, dv]; one stationary serves both dv tiles
                for t in range(DC):
                    psl = [mm_ps.tile([P, NT], f32, tag="mm", name="ps")
                           for _ in range(D // NT)]
                    for dc in range(DC):
                        for j in range(D // NT):
                            nc.tensor.matmul(
                                psl[j][:],
                                XQ_sb[:, dc, t * P:(t + 1) * P],
                                WV_sb[:, dc, j * NT:(j + 1) * NT],
                                start=(dc == 0),
                                stop=(dc == DC - 1),
                            )
                    for j in range(D // NT):
                        nc.vector.tensor_copy(
                            out=VL_sb[:, t, j * NT:(j + 1) * NT], in_=psl[j][:])
                v_in = dram_pool.tile([P, DC, SH], bf16, name="v_in")
                v_out = dram_pool.tile([2, P, DC, SH], bf16, name="v_out")
                nc.sync.dma_start(v_in[:], VL_sb[:])
                nc.gpsimd.collective_compute(
                    "AllGather", mybir.AluOpType.bypass,
                    replica_groups=GROUPS,
                    ins=[v_in.opt()], outs=[v_out.opt()])
                nc.sync.dma_start(V_sb[:, 0:DC, :], v_out[0])
                nc.sync.dma_start(V_sb[:, DC:TC, :], v_out[1])
            # ---- phase B/C: scores -> softmax -> +I^T -> attn^T -------
            AT_sb = persist.tile([P, TC, SH], bf16, tag="xq_at")  # attn^T
            with (
                tc.tile_pool(name="e_pool", bufs=2) as e_pool,
                tc.tile_pool(name="it_pool", bufs=3) as it_pool,
                tc.tile_pool(name="stat_pool", bufs=2) as stat_pool,
            ):
                for si in range(DC):  # 8 chunks of 128 own query rows
                    E_sb = e_pool.tile([P, S], bf16, tag="e")
                    acc4 = stat_pool.tile([P, TJ], f32, tag="acc")
                    IT_sb = it_pool.tile([P, S], bf16, tag="it")
                    nc.sync.dma_start(IT_sb[:], it_v[si])
                    # First two si chunks run tj-sequential so the PE can
                    # restart on the first gathered K block without waiting
                    # for the second AllGather; later chunks interleave tj
                    # across psum banks to share the stationary operand.
                    if si < 2:
                        for tj in range(TJ):
                            ps = mm_ps.tile([P, NT], f32, tag="mm", name="ps")
                            for dc in range(DC):
                                nc.tensor.matmul(
                                    ps[:],
                                    QT_sb[:, dc, si * P:(si + 1) * P],
                                    KT_sb[:, tj // 2, tj % 2, dc, :],
                                    start=(dc == 0),
                                    stop=(dc == DC - 1),
                                )
                            nc.scalar.activation(
                                E_sb[:, tj * NT:(tj + 1) * NT], ps[:], Exp,
                                scale=SCALE, accum_out=acc4[:, tj:tj + 1],
                            )
                    else:
                        psl = [mm_ps.tile([P, NT], f32, tag="mm", name="ps")
                               for _ in range(TJ)]
                        for dc in range(DC):
                            for tj in range(TJ):
                                nc.tensor.matmul(
                                    psl[tj][:],
                                    QT_sb[:, dc, si * P:(si + 1) * P],
                                    KT_sb[:, tj // 2, tj % 2, dc, :],
                                    start=(dc == 0),
                                    stop=(dc == DC - 1),
                                )
                        for tj in range(TJ):
                            nc.scalar.activation(
                                E_sb[:, tj * NT:(tj + 1) * NT], psl[tj][:], Exp,
                                scale=SCALE, accum_out=acc4[:, tj:tj + 1],
                            )
                    den = stat_pool.tile([P, 1], f32, tag="den")
                    recip = stat_pool.tile([P, 1], f32, tag="recip")
                    diag = stat_pool.tile([P, P], bf16, tag="diag")
                    nc.vector.reduce_sum(den[:], acc4[:], axis=mybir.AxisListType.X)
                    nc.vector.reciprocal(recip[:], den[:])
                    # diag(recip): identity rows scaled per-partition
                    nc.vector.tensor_scalar_mul(diag[:], ident[:], recip[:])
                    # attn^T tile = E-slice.T @ diag  (transpose + normalize),
                    # then the PSUM drain adds intensity^T on DVE.
                    for t in range(TC):
                        pt = tr_ps.tile([P, P], f32, tag="tr")
                        nc.tensor.matmul(
                            pt[:], E_sb[:, t * P:(t + 1) * P], diag[:],
                            start=True, stop=True)
                        nc.vector.tensor_tensor(
                            AT_sb[:, t, si * P:(si + 1) * P],
                            pt[:], IT_sb[:, t * P:(t + 1) * P], add)

            # ---- phase D/E: PV -> out^T, then projection per s-tile ---
            OT_sb = persist.tile([P, DC, SH], bf16, tag="qt_ot")
            with (
                tc.tile_pool(name="wo_pool", bufs=1) as wo_pool,
                tc.tile_pool(name="fin_pool", bufs=3) as fin_pool,
            ):
                WO_sb = wo_pool.tile([P, DC, D], bf16)
                nc.sync.dma_start(WO_sb[:], wo_v)
                for sj in range(SJ):
                    for dvi in range(DC):
                        ps = mm_ps.tile([P, NT], f32, tag="mm")
                        for t in range(TC):
                            nc.tensor.matmul(
                                ps[:],
                                V_sb[:, t, dvi * P:(dvi + 1) * P],
                                AT_sb[:, t, sj * NT:(sj + 1) * NT],
                                start=(t == 0),
                                stop=False,
                            )
                        # bias: bv (x) (1 + rowsum(I))
                        nc.tensor.matmul(
                            ps[:], BV_sb[0:1, dvi * P:(dvi + 1) * P],
                            RS_sb[0:1, sj * NT:(sj + 1) * NT],
                            start=False, stop=True)
                        nc.vector.tensor_copy(
                            out=OT_sb[:, dvi, sj * NT:(sj + 1) * NT], in_=ps[:])

                    for doi in range(DC):
                        ps = mm_ps.tile([P, NT], f32, tag="mm")
                        for dvc in range(DC):
                            nc.tensor.matmul(
                                ps[:],
                                WO_sb[:, dvc, doi * P:(doi + 1) * P],
                                OT_sb[:, dvc, sj * NT:(sj + 1) * NT],
                                start=(dvc == 0), stop=(dvc == DC - 1),
                            )
                        F_sb = fin_pool.tile([P, NT], f32, tag="fin")
                        nc.vector.tensor_scalar_add(
                            F_sb[:], ps[:],
                            BCOL_sb[:, 2 * DC + doi:2 * DC + doi + 1])
                        nc.sync.dma_start(
                            out_v[:, doi, sj * NT:(sj + 1) * NT], F_sb[:])

    nc.compile()
    return nc


def _get_module():
    if "nc" not in _CACHE:
        _CACHE["nc"] = _build_module()
    return _CACHE["nc"]


def _make_in_maps(inputs):
    X = np.asarray(inputs["X"], dtype=np.float32)
    intensity = np.asarray(inputs["intensity"], dtype=np.float32)
    bf = ml_dtypes.bfloat16
    WqT = np.ascontiguousarray(np.asarray(inputs["Wq"], np.float32).T).astype(bf)
    WkT = np.ascontiguousarray(np.asarray(inputs["Wk"], np.float32).T).astype(bf)
    WvT = np.ascontiguousarray(np.asarray(inputs["Wv"], np.float32).T).astype(bf)
    WoT = np.ascontiguousarray(np.asarray(inputs["Wo"], np.float32).T).astype(bf)
    bq, bk, bv, bo = (np.asarray(inputs[k], np.float32).reshape(D)
                      for k in ("bq", "bk", "bv", "bo"))
    BCOL = np.concatenate(
        [b.reshape(DC, P).T for b in (bq, bk, bo)], axis=1
    ).astype(np.float32)  # [128, 24]

    in_maps = []
    for c in range(8):
        b, h = c // 2, c % 2
        XQT = np.ascontiguousarray(X[b, h * SH:(h + 1) * SH, :].T).astype(bf)
        Islc = intensity[b, h * SH:(h + 1) * SH, :]
        # [t, s] -> [si*128+tp, tc*128+sp] so each per-si load is one
        # contiguous row-block (128 descriptors instead of 2048)
        IT = np.ascontiguousarray(
            Islc.T.reshape(TC, P, DC, P).transpose(2, 1, 0, 3).reshape(SH, S)
        ).astype(bf)
        rows = 1.0 + Islc.sum(axis=1, dtype=np.float64).astype(np.float32)
        BROW = np.concatenate([bv, rows]).reshape(1, D + SH)
        in_maps.append({
            "XQT": XQT, "WQT": WqT, "WKT": WkT, "WVT": WvT, "WOT": WoT,
            "BCOL": BCOL, "BROW": BROW, "IT": IT,
        })
    return in_maps


def _gather(results):
    out = np.empty((4, S, D), dtype=np.float32)
    for c in range(8):
        b, h = c // 2, c % 2
        out[b, h * SH:(h + 1) * SH, :] = results[c]["OUTT"].T
    return out


def kernel(**inputs):
    from concourse import bass_utils

    in_maps = _make_in_maps(inputs)
    nc = _get_module()
    res = bass_utils.run_bass_kernel_spmd(nc, in_maps, core_ids=list(range(8)))
    return _gather(res.results)


# revision 26
# speedup vs baseline: 1.1491x; 1.1491x over previous
"""Trainium2 Bass kernel for nn_Attention_54254026883778.

Single-head attention with an additive post-softmax intensity term:
    q/k/v = X @ W{q,k,v}.T + b;  scores = q k^T / sqrt(D)
    attn  = softmax(scores) + intensity;  out = (attn @ v) @ Wo.T + bo

Sharding: 8 cores = 4 batches x 2 sequence halves. Each core computes
K^T and V for its whole batch (duplicated across the pair) and Q/attention
for its own 1024 query rows. No collectives. The host rolls the t-axis per
core so the core's own query rows are always at t-positions 0..SH; K/V and
intensity follow the same rolled order, which leaves attn @ V invariant.

Device dataflow (host pre-transposes X, W and intensity so contraction /
partition dims land where the engines want them):
    Q^T/K^T  [dout | s]  = WxT-chunk.T @ XT        (lhsT=WxT, rhs=XT)
    V        [t | dv]    = XT-chunk.T @ WvT        (lhsT=XT,  rhs=WvT)
    scores   [s | t]     = QT-chunk.T @ KT  -> exp on ACT (no max-subtract:
        |scores| < ~3) with fused row-accumulate -> 1/den on DVE ->
        diag(recip) = ident * recip (DVE) ->
        attn^T tile = E-slice.T @ diag(recip)      (one PE matmul both
        transposes and normalizes) -> DVE copy adds intensity^T (bf16,
        host-transposed) while draining PSUM -> attn^T [t | s]
    out^T    [dv | s]    = V-chunk.T @ attn^T
    final^T  [do | s]    = WoT-chunk.T @ out^T     -> DRAM, host transposes
Biases enter as rank-1 fp32r matmuls accumulated into PSUM:
    q/k/o:  b_row (x) ones_row;   v: bv_row (x) attn-rowsums, where the
    rowsums are 1 + rowsum(intensity) (softmax rows sum to 1), shipped
    from the host inside the bias pack.
"""

import numpy as np
import ml_dtypes

P = 128
D = 1024
S = 2048          # keys per batch (full sequence)
SH = 1024         # query rows owned by each core
DC = D // P       # 8  contraction chunks over model dim
TC = S // P       # 16 t (key) chunks
NT = 512          # matmul moving free dim / psum bank
SJ = SH // NT     # 2  s-tiles of own rows
TJ = S // NT      # 4  t-tiles
SCALE = 1.0 / 32.0  # 1/sqrt(D)

_CACHE = {}


def _build_module():
    import concourse.bass as bass
    import concourse.tile as tile
    import concourse.mybir as mybir
    from concourse import bacc
    from concourse.masks import make_identity

    f32 = mybir.dt.float32
    f32r = mybir.dt.float32r
    bf16 = mybir.dt.bfloat16
    Exp = mybir.ActivationFunctionType.Exp
    add = mybir.AluOpType.add

    nc = bacc.Bacc("TRN2", target_bir_lowering=False, debug=False,
                   num_devices=8)

    XQ_d = nc.dram_tensor("XQT", [D, SH], bf16, kind="ExternalInput")
    WQ_d = nc.dram_tensor("WQT", [D, D], bf16, kind="ExternalInput")
    WK_d = nc.dram_tensor("WKT", [D, D], bf16, kind="ExternalInput")
    WV_d = nc.dram_tensor("WVT", [D, D], bf16, kind="ExternalInput")
    WO_d = nc.dram_tensor("WOT", [D, D], bf16, kind="ExternalInput")
    BCOL_d = nc.dram_tensor("BCOL", [P, 3 * DC], f32, kind="ExternalInput")
    BROW_d = nc.dram_tensor("BROW", [1, D + SH], f32, kind="ExternalInput")
    IT_d = nc.dram_tensor("IT", [SH, S], bf16, kind="ExternalInput")
    OUT_d = nc.dram_tensor("OUTT", [D, SH], f32, kind="ExternalOutput")

    xq_v = XQ_d[:].rearrange("(c p) s -> p c s", p=P)
    wq_v = WQ_d[:].rearrange("(c p) o -> p c o", p=P)
    wk_v = WK_d[:].rearrange("(c p) o -> p c o", p=P)
    wv_v = WV_d[:].rearrange("(c p) o -> p c o", p=P)
    wo_v = WO_d[:].rearrange("(c p) o -> p c o", p=P)
    it_v = IT_d[:].rearrange("(si p) f -> si p f", p=P)  # per-si blocks
    out_v = OUT_d[:].rearrange("(c p) s -> p c s", p=P)

    with tile.TileContext(nc) as tc:
        with (
            tc.tile_pool(name="persist", bufs=1) as persist,
            tc.tile_pool(name="mm_ps", bufs=4, space="PSUM") as mm_ps,
            tc.tile_pool(name="tr_ps", bufs=4, space="PSUM") as tr_ps,
            tc.tile_pool(name="dram", bufs=1, space="DRAM") as dram_pool,
        ):
            GROUPS = [[0, 1], [2, 3], [4, 5], [6, 7]]
            # ---- persistent tiles -------------------------------------
            KT_sb = persist.tile([P, 2, SJ, DC, NT], bf16)  # K^T [d | blk,tt]
            V_sb = persist.tile([P, TC, D], bf16)           # V natural [t | dv]
            QT_sb = persist.tile([P, DC, SH], bf16, tag="qt_ot")   # Q^T [d | s]

            ident = persist.tile([P, P], bf16)
            make_identity(nc, ident)
            # bq|bk|bo as per-partition columns, added during PSUM extract
            BCOL_sb = persist.tile([P, 3 * DC], f32)
            nc.sync.dma_start(BCOL_sb[:], BCOL_d[:])
            # bv and the attn rowsums feed the PV rank-1 bias matmul; fp32r
            # operands must come from a rounding instruction, so stage the
            # DMA through a DVE copy.
            BROW_ld = persist.tile([1, D + SH], f32)
            nc.sync.dma_start(BROW_ld[:], BROW_d[:])
            BROW_r = persist.tile([1, D + SH], f32r)
            nc.vector.tensor_copy(out=BROW_r[:], in_=BROW_ld[:])
            BV_sb = BROW_r[0:1, 0:D]
            RS_sb = BROW_r[0:1, D:D + SH]                   # 1 + rowsum(I)

            # ---- phase A: QKV (local rows) + pair AllGather of K/V ----
            # Each core computes Q^T, and K^T/V for ITS OWN 1024 rows only
            # (from XQT); the partner's half arrives via a 2-rank AllGather
            # through shared DRAM, placed in global t-order (SPMD-static).
            XQ_sb = persist.tile([P, DC, SH], bf16, tag="xq_at")
            KL_sb = persist.tile([P, SJ, DC, NT], bf16)  # local K^T [tt | d]
            VL_sb = persist.tile([P, DC, SH], bf16)      # local V [t-own | dv]
            with tc.tile_pool(name="wpool", bufs=2) as wpool:
                WK_sb = wpool.tile([P, DC, D], bf16, tag="w")
                WQ_sb = wpool.tile([P, DC, D], bf16, tag="w")
                # Per-chunk DMAs, K operands first: K-local runs first so its
                # AllGather kicks off as early as possible and hides under
                # the Q and V-local compute.
                for dc in range(DC):
                    nc.sync.dma_start(XQ_sb[:, dc, :], xq_v[:, dc, :])
                    nc.sync.dma_start(WK_sb[:, dc, :], wk_v[:, dc, :])
                for dc in range(DC):
                    nc.sync.dma_start(WQ_sb[:, dc, :], wq_v[:, dc, :])

                # K^T local [dout, t-own], t-tile outer so each half's
                # AllGather kicks off as soon as that half is computed
                for j in range(SJ):
                    for c in range(DC):
                        ps = mm_ps.tile([P, NT], f32, tag="mm", name="ps")
                        for dc in range(DC):
                            nc.tensor.matmul(
                                ps[:],
                                WK_sb[:, dc, c * P:(c + 1) * P],
                                XQ_sb[:, dc, j * NT:(j + 1) * NT],
                                start=(dc == 0), stop=(dc == DC - 1),
                            )
                        nc.vector.tensor_scalar_add(
                            KL_sb[:, j, c, :], ps[:],
                            BCOL_sb[:, DC + c:DC + c + 1])
                    k_in = dram_pool.tile([P, DC, NT], bf16, name="k_in")
                    k_out = dram_pool.tile([2, P, DC, NT], bf16, name="k_out")
                    nc.sync.dma_start(k_in[:], KL_sb[:, j])
                    nc.gpsimd.collective_compute(
                        "AllGather", mybir.AluOpType.bypass,
                        replica_groups=GROUPS,
                        ins=[k_in.opt()], outs=[k_out.opt()])
                    nc.sync.dma_start(KT_sb[:, 0, j], k_out[0])
                    nc.sync.dma_start(KT_sb[:, 1, j], k_out[1])

                # Q^T [dout, s-own]
                for c in range(DC):
                    psl = [mm_ps.tile([P, NT], f32, tag="mm", name="ps")
                           for _ in range(SJ)]
                    for dc in range(DC):
                        for j in range(SJ):
                            nc.tensor.matmul(
                                psl[j][:],
                                WQ_sb[:, dc, c * P:(c + 1) * P],
                                XQ_sb[:, dc, j * NT:(j + 1) * NT],
                                start=(dc == 0), stop=(dc == DC - 1),
                            )
                    for j in range(SJ):
                        nc.vector.tensor_scalar_add(
                            QT_sb[:, c, j * NT:(j + 1) * NT], psl[j][:],
                            BCOL_sb[:, c:c + 1])

                WV_sb = wpool.tile([P, DC, D], bf16, tag="w")
                for dc in range(DC):
                    nc.sync.dma_start(WV_sb[:, dc, :], wv_v[:, dc, :])
                # V local [t-own, dv]; one stationary serves both dv tiles
                for t in range(DC):
                    psl = [mm_ps.tile([P, NT], f32, tag="mm", name="ps")
                           for _ in range(D // NT)]
                    for dc in range(DC):
                        for j in range(D // NT):
                            nc.tensor.matmul(
                                psl[j][:],
                                XQ_sb[:, dc, t * P:(t + 1) * P],
                                WV_sb[:, dc, j * NT:(j + 1) * NT],
                                start=(dc == 0),
                                stop=(dc == DC - 1),
                            )
                    for j in range(D // NT):
                        nc.vector.tensor_copy(
                            out=VL_sb[:, t, j * NT:(j + 1) * NT], in_=psl[j][:])
                v_in = dram_pool.tile([P, DC, SH], bf16, name="v_in")
                v_out = dram_pool.tile([2, P, DC, SH], bf16, name="v_out")
                nc.sync.dma_start(v_in[:], VL_sb[:])
                nc.gpsimd.collective_compute(
                    "AllGather", mybir.AluOpType.bypass,
                    replica_groups=GROUPS,
                    ins=[v_in.opt()], outs=[v_out.opt()])
                nc.sync.dma_start(V_sb[:, 0:DC, :], v_out[0])
                nc.sync.dma_start(V_sb[:, DC:TC, :], v_out[1])
            # ---- phase B/C: scores -> softmax -> +I^T -> attn^T -------
            AT_sb = persist.tile([P, TC, SH], bf16, tag="xq_at")  # attn^T
            with (
                tc.tile_pool(name="e_pool", bufs=2) as e_pool,
                tc.tile_pool(name="it_pool", bufs=3) as it_pool,
                tc.tile_pool(name="stat_pool", bufs=2) as stat_pool,
            ):
                for si in range(DC):  # 8 chunks of 128 own query rows
                    E_sb = e_pool.tile([P, S], bf16, tag="e")
                    acc4 = stat_pool.tile([P, TJ], f32, tag="acc")
                    IT_sb = it_pool.tile([P, S], bf16, tag="it")
                    nc.sync.dma_start(IT_sb[:], it_v[si])
                    psl = [mm_ps.tile([P, NT], f32, tag="mm", name="ps")
                           for _ in range(TJ)]
                    for dc in range(DC):
                        for tj in range(TJ):
                            nc.tensor.matmul(
                                psl[tj][:],
                                QT_sb[:, dc, si * P:(si + 1) * P],
                                KT_sb[:, tj // 2, tj % 2, dc, :],
                                start=(dc == 0),
                                stop=(dc == DC - 1),
                            )
                    for tj in range(TJ):
                        nc.scalar.activation(
                            E_sb[:, tj * NT:(tj + 1) * NT], psl[tj][:], Exp,
                            scale=SCALE, accum_out=acc4[:, tj:tj + 1],
                        )
                    den = stat_pool.tile([P, 1], f32, tag="den")
                    recip = stat_pool.tile([P, 1], f32, tag="recip")
                    diag = stat_pool.tile([P, P], bf16, tag="diag")
                    nc.vector.reduce_sum(den[:], acc4[:], axis=mybir.AxisListType.X)
                    nc.vector.reciprocal(recip[:], den[:])
                    # diag(recip): identity rows scaled per-partition
                    nc.vector.tensor_scalar_mul(diag[:], ident[:], recip[:])
                    # attn^T tile = E-slice.T @ diag  (transpose + normalize),
                    # then the PSUM drain adds intensity^T on DVE.
                    for t in range(TC):
                        pt = tr_ps.tile([P, P], f32, tag="tr")
                        nc.tensor.matmul(
                            pt[:], E_sb[:, t * P:(t + 1) * P], diag[:],
                            start=True, stop=True)
                        nc.vector.tensor_tensor(
                            AT_sb[:, t, si * P:(si + 1) * P],
                            pt[:], IT_sb[:, t * P:(t + 1) * P], add)

            # ---- phase D/E: PV -> out^T, then projection per s-tile ---
            OT_sb = persist.tile([P, DC, SH], bf16, tag="qt_ot")
            with (
                tc.tile_pool(name="wo_pool", bufs=1) as wo_pool,
                tc.tile_pool(name="fin_pool", bufs=3) as fin_pool,
            ):
                WO_sb = wo_pool.tile([P, DC, D], bf16)
                nc.sync.dma_start(WO_sb[:], wo_v)
                for sj in range(SJ):
                    for dvi in range(DC):
                        ps = mm_ps.tile([P, NT], f32, tag="mm")
                        for t in range(TC):
                            nc.tensor.matmul(
                                ps[:],
                                V_sb[:, t, dvi * P:(dvi + 1) * P],
                                AT_sb[:, t, sj * NT:(sj + 1) * NT],
                                start=(t == 0),
                                stop=False,
                            )
                        # bias: bv (x) (1 + rowsum(I))
                        nc.tensor.matmul(
                            ps[:], BV_sb[0:1, dvi * P:(dvi + 1) * P],
                            RS_sb[0:1, sj * NT:(sj + 1) * NT],
                            start=False, stop=True)
                        nc.vector.tensor_copy(
                            out=OT_sb[:, dvi, sj * NT:(sj + 1) * NT], in_=ps[:])

                    for doi in range(DC):
                        ps = mm_ps.tile([P, NT], f32, tag="mm")
                        for dvc in range(DC):
                            nc.tensor.matmul(
                                ps[:],
                                WO_sb[:, dvc, doi * P:(doi + 1) * P],
                                OT_sb[:, dvc, sj * NT:(sj + 1) * NT],
                                start=(dvc == 0), stop=(dvc == DC - 1),
                            )
                        F_sb = fin_pool.tile([P, NT], f32, tag="fin")
                        nc.vector.tensor_scalar_add(
                            F_sb[:], ps[:],
                            BCOL_sb[:, 2 * DC + doi:2 * DC + doi + 1])
                        nc.sync.dma_start(
                            out_v[:, doi, sj * NT:(sj + 1) * NT], F_sb[:])

    nc.compile()
    return nc


def _get_module():
    if "nc" not in _CACHE:
        _CACHE["nc"] = _build_module()
    return _CACHE["nc"]


def _make_in_maps(inputs):
    X = np.asarray(inputs["X"], dtype=np.float32)
    intensity = np.asarray(inputs["intensity"], dtype=np.float32)
    bf = ml_dtypes.bfloat16
    WqT = np.ascontiguousarray(np.asarray(inputs["Wq"], np.float32).T).astype(bf)
    WkT = np.ascontiguousarray(np.asarray(inputs["Wk"], np.float32).T).astype(bf)
    WvT = np.ascontiguousarray(np.asarray(inputs["Wv"], np.float32).T).astype(bf)
    WoT = np.ascontiguousarray(np.asarray(inputs["Wo"], np.float32).T).astype(bf)
    bq, bk, bv, bo = (np.asarray(inputs[k], np.float32).reshape(D)
                      for k in ("bq", "bk", "bv", "bo"))
    BCOL = np.concatenate(
        [b.reshape(DC, P).T for b in (bq, bk, bo)], axis=1
    ).astype(np.float32)  # [128, 24]

    in_maps = []
    for c in range(8):
        b, h = c // 2, c % 2
        XQT = np.ascontiguousarray(X[b, h * SH:(h + 1) * SH, :].T).astype(bf)
        Islc = intensity[b, h * SH:(h + 1) * SH, :]
        # [t, s] -> [si*128+tp, tc*128+sp] so each per-si load is one
        # contiguous row-block (128 descriptors instead of 2048)
        IT = np.ascontiguousarray(
            Islc.T.reshape(TC, P, DC, P).transpose(2, 1, 0, 3).reshape(SH, S)
        ).astype(bf)
        rows = 1.0 + Islc.sum(axis=1, dtype=np.float64).astype(np.float32)
        BROW = np.concatenate([bv, rows]).reshape(1, D + SH)
        in_maps.append({
            "XQT": XQT, "WQT": WqT, "WKT": WkT, "WVT": WvT, "WOT": WoT,
            "BCOL": BCOL, "BROW": BROW, "IT": IT,
        })
    return in_maps


def _gather(results):
    out = np.empty((4, S, D), dtype=np.float32)
    for c in range(8):
        b, h = c // 2, c % 2
        out[b, h * SH:(h + 1) * SH, :] = results[c]["OUTT"].T
    return out


def kernel(**inputs):
    from concourse import bass_utils

    in_maps = _make_in_maps(inputs)
    nc = _get_module()
    res = bass_utils.run_bass_kernel_spmd(nc, in_maps, core_ids=list(range(8)))
    return _gather(res.results)


# revision 27
# speedup vs baseline: 1.1567x; 1.0066x over previous
"""Trainium2 Bass kernel for nn_Attention_54254026883778.

Single-head attention with an additive post-softmax intensity term:
    q/k/v = X @ W{q,k,v}.T + b;  scores = q k^T / sqrt(D)
    attn  = softmax(scores) + intensity;  out = (attn @ v) @ Wo.T + bo

Sharding: 8 cores = 4 batches x 2 sequence halves. Each core computes
K^T and V for its whole batch (duplicated across the pair) and Q/attention
for its own 1024 query rows. No collectives. The host rolls the t-axis per
core so the core's own query rows are always at t-positions 0..SH; K/V and
intensity follow the same rolled order, which leaves attn @ V invariant.

Device dataflow (host pre-transposes X, W and intensity so contraction /
partition dims land where the engines want them):
    Q^T/K^T  [dout | s]  = WxT-chunk.T @ XT        (lhsT=WxT, rhs=XT)
    V        [t | dv]    = XT-chunk.T @ WvT        (lhsT=XT,  rhs=WvT)
    scores   [s | t]     = QT-chunk.T @ KT  -> exp on ACT (no max-subtract:
        |scores| < ~3) with fused row-accumulate -> 1/den on DVE ->
        diag(recip) = ident * recip (DVE) ->
        attn^T tile = E-slice.T @ diag(recip)      (one PE matmul both
        transposes and normalizes) -> DVE copy adds intensity^T (bf16,
        host-transposed) while draining PSUM -> attn^T [t | s]
    out^T    [dv | s]    = V-chunk.T @ attn^T
    final^T  [do | s]    = WoT-chunk.T @ out^T     -> DRAM, host transposes
Biases enter as rank-1 fp32r matmuls accumulated into PSUM:
    q/k/o:  b_row (x) ones_row;   v: bv_row (x) attn-rowsums, where the
    rowsums are 1 + rowsum(intensity) (softmax rows sum to 1), shipped
    from the host inside the bias pack.
"""

import numpy as np
import ml_dtypes

P = 128
D = 1024
S = 2048          # keys per batch (full sequence)
SH = 1024         # query rows owned by each core
DC = D // P       # 8  contraction chunks over model dim
TC = S // P       # 16 t (key) chunks
NT = 512          # matmul moving free dim / psum bank
SJ = SH // NT     # 2  s-tiles of own rows
TJ = S // NT      # 4  t-tiles
SCALE = 1.0 / 32.0  # 1/sqrt(D)

_CACHE = {}


def _build_module():
    import concourse.bass as bass
    import concourse.tile as tile
    import concourse.mybir as mybir
    from concourse import bacc
    from concourse.masks import make_identity

    f32 = mybir.dt.float32
    f32r = mybir.dt.float32r
    bf16 = mybir.dt.bfloat16
    Exp = mybir.ActivationFunctionType.Exp
    add = mybir.AluOpType.add

    nc = bacc.Bacc("TRN2", target_bir_lowering=False, debug=False,
                   num_devices=8)

    XQ_d = nc.dram_tensor("XQT", [D, SH], bf16, kind="ExternalInput")
    WQ_d = nc.dram_tensor("WQT", [D, D], bf16, kind="ExternalInput")
    WK_d = nc.dram_tensor("WKT", [D, D], bf16, kind="ExternalInput")
    WV_d = nc.dram_tensor("WVT", [D, D], bf16, kind="ExternalInput")
    WO_d = nc.dram_tensor("WOT", [D, D], bf16, kind="ExternalInput")
    BCOL_d = nc.dram_tensor("BCOL", [P, 3 * DC], f32, kind="ExternalInput")
    BROW_d = nc.dram_tensor("BROW", [1, D + SH], f32, kind="ExternalInput")
    IT_d = nc.dram_tensor("IT", [SH, S], bf16, kind="ExternalInput")
    OUT_d = nc.dram_tensor("OUTT", [D, SH], f32, kind="ExternalOutput")

    xq_v = XQ_d[:].rearrange("(c p) s -> p c s", p=P)
    wq_v = WQ_d[:].rearrange("(c p) o -> p c o", p=P)
    wk_v = WK_d[:].rearrange("(c p) o -> p c o", p=P)
    wv_v = WV_d[:].rearrange("(c p) o -> p c o", p=P)
    wo_v = WO_d[:].rearrange("(c p) o -> p c o", p=P)
    it_v = IT_d[:].rearrange("(si p) f -> si p f", p=P)  # per-si blocks
    out_v = OUT_d[:].rearrange("(c p) s -> p c s", p=P)

    with tile.TileContext(nc) as tc:
        with (
            tc.tile_pool(name="persist", bufs=1) as persist,
            tc.tile_pool(name="mm_ps", bufs=5, space="PSUM") as mm_ps,
            tc.tile_pool(name="tr_ps", bufs=3, space="PSUM") as tr_ps,
            tc.tile_pool(name="dram", bufs=1, space="DRAM") as dram_pool,
        ):
            GROUPS = [[0, 1], [2, 3], [4, 5], [6, 7]]
            # ---- persistent tiles -------------------------------------
            KT_sb = persist.tile([P, 2, SJ, DC, NT], bf16)  # K^T [d | blk,tt]
            V_sb = persist.tile([P, TC, D], bf16)           # V natural [t | dv]
            QT_sb = persist.tile([P, DC, SH], bf16, tag="qt_ot")   # Q^T [d | s]

            ident = persist.tile([P, P], bf16)
            make_identity(nc, ident)
            # bq|bk|bo as per-partition columns, added during PSUM extract
            BCOL_sb = persist.tile([P, 3 * DC], f32)
            nc.sync.dma_start(BCOL_sb[:], BCOL_d[:])
            # bv and the attn rowsums feed the PV rank-1 bias matmul; fp32r
            # operands must come from a rounding instruction, so stage the
            # DMA through a DVE copy.
            BROW_ld = persist.tile([1, D + SH], f32)
            nc.sync.dma_start(BROW_ld[:], BROW_d[:])
            BROW_r = persist.tile([1, D + SH], f32r)
            nc.vector.tensor_copy(out=BROW_r[:], in_=BROW_ld[:])
            BV_sb = BROW_r[0:1, 0:D]
            RS_sb = BROW_r[0:1, D:D + SH]                   # 1 + rowsum(I)

            # ---- phase A: QKV (local rows) + pair AllGather of K/V ----
            # Each core computes Q^T, and K^T/V for ITS OWN 1024 rows only
            # (from XQT); the partner's half arrives via a 2-rank AllGather
            # through shared DRAM, placed in global t-order (SPMD-static).
            XQ_sb = persist.tile([P, DC, SH], bf16, tag="xq_at")
            KL_sb = persist.tile([P, SJ, DC, NT], bf16)  # local K^T [tt | d]
            VL_sb = persist.tile([P, DC, SH], bf16)      # local V [t-own | dv]
            with tc.tile_pool(name="wpool", bufs=2) as wpool:
                WK_sb = wpool.tile([P, DC, D], bf16, tag="w")
                WQ_sb = wpool.tile([P, DC, D], bf16, tag="w")
                # Per-chunk DMAs, K operands first: K-local runs first so its
                # AllGather kicks off as early as possible and hides under
                # the Q and V-local compute.
                for dc in range(DC):
                    nc.sync.dma_start(XQ_sb[:, dc, :], xq_v[:, dc, :])
                    nc.sync.dma_start(WK_sb[:, dc, :], wk_v[:, dc, :])
                for dc in range(DC):
                    nc.sync.dma_start(WQ_sb[:, dc, :], wq_v[:, dc, :])

                # K^T local [dout, t-own], t-tile outer so each half's
                # AllGather kicks off as soon as that half is computed
                for j in range(SJ):
                    for c in range(DC):
                        ps = mm_ps.tile([P, NT], f32, tag="mm", name="ps")
                        for dc in range(DC):
                            nc.tensor.matmul(
                                ps[:],
                                WK_sb[:, dc, c * P:(c + 1) * P],
                                XQ_sb[:, dc, j * NT:(j + 1) * NT],
                                start=(dc == 0), stop=(dc == DC - 1),
                            )
                        nc.vector.tensor_scalar_add(
                            KL_sb[:, j, c, :], ps[:],
                            BCOL_sb[:, DC + c:DC + c + 1])
                    k_in = dram_pool.tile([P, DC, NT], bf16, name="k_in")
                    k_out = dram_pool.tile([2, P, DC, NT], bf16, name="k_out")
                    nc.gpsimd.dma_start(k_in[:], KL_sb[:, j])
                    nc.gpsimd.collective_compute(
                        "AllGather", mybir.AluOpType.bypass,
                        replica_groups=GROUPS,
                        ins=[k_in.opt()], outs=[k_out.opt()])
                    nc.gpsimd.dma_start(KT_sb[:, 0, j], k_out[0])
                    nc.gpsimd.dma_start(KT_sb[:, 1, j], k_out[1])

                # Q^T [dout, s-own]
                for c in range(DC):
                    psl = [mm_ps.tile([P, NT], f32, tag="mm", name="ps")
                           for _ in range(SJ)]
                    for dc in range(DC):
                        for j in range(SJ):
                            nc.tensor.matmul(
                                psl[j][:],
                                WQ_sb[:, dc, c * P:(c + 1) * P],
                                XQ_sb[:, dc, j * NT:(j + 1) * NT],
                                start=(dc == 0), stop=(dc == DC - 1),
                            )
                    for j in range(SJ):
                        nc.vector.tensor_scalar_add(
                            QT_sb[:, c, j * NT:(j + 1) * NT], psl[j][:],
                            BCOL_sb[:, c:c + 1])

                WV_sb = wpool.tile([P, DC, D], bf16, tag="w")
                for dc in range(DC):
                    nc.sync.dma_start(WV_sb[:, dc, :], wv_v[:, dc, :])
                # V local [t-own, dv]; one stationary serves both dv tiles
                for t in range(DC):
                    psl = [mm_ps.tile([P, NT], f32, tag="mm", name="ps")
                           for _ in range(D // NT)]
                    for dc in range(DC):
                        for j in range(D // NT):
                            nc.tensor.matmul(
                                psl[j][:],
                                XQ_sb[:, dc, t * P:(t + 1) * P],
                                WV_sb[:, dc, j * NT:(j + 1) * NT],
                                start=(dc == 0),
                                stop=(dc == DC - 1),
                            )
                    for j in range(D // NT):
                        nc.vector.tensor_copy(
                            out=VL_sb[:, t, j * NT:(j + 1) * NT], in_=psl[j][:])
                v_in = dram_pool.tile([P, DC, SH], bf16, name="v_in")
                v_out = dram_pool.tile([2, P, DC, SH], bf16, name="v_out")
                nc.sync.dma_start(v_in[:], VL_sb[:])
                nc.gpsimd.collective_compute(
                    "AllGather", mybir.AluOpType.bypass,
                    replica_groups=GROUPS,
                    ins=[v_in.opt()], outs=[v_out.opt()])
                nc.sync.dma_start(V_sb[:, 0:DC, :], v_out[0])
                nc.sync.dma_start(V_sb[:, DC:TC, :], v_out[1])
            # ---- phase B/C: scores -> softmax -> +I^T -> attn^T -------
            AT_sb = persist.tile([P, TC, SH], bf16, tag="xq_at")  # attn^T
            with (
                tc.tile_pool(name="e_pool", bufs=2) as e_pool,
                tc.tile_pool(name="it_pool", bufs=3) as it_pool,
                tc.tile_pool(name="stat_pool", bufs=2) as stat_pool,
            ):
                for si in range(DC):  # 8 chunks of 128 own query rows
                    E_sb = e_pool.tile([P, S], bf16, tag="e")
                    acc4 = stat_pool.tile([P, TJ], f32, tag="acc")
                    IT_sb = it_pool.tile([P, S], bf16, tag="it")
                    nc.sync.dma_start(IT_sb[:], it_v[si])
                    psl = [mm_ps.tile([P, NT], f32, tag="mm", name="ps")
                           for _ in range(TJ)]
                    for dc in range(DC):
                        for tj in range(TJ):
                            nc.tensor.matmul(
                                psl[tj][:],
                                QT_sb[:, dc, si * P:(si + 1) * P],
                                KT_sb[:, tj // 2, tj % 2, dc, :],
                                start=(dc == 0),
                                stop=(dc == DC - 1),
                            )
                    for tj in range(TJ):
                        nc.scalar.activation(
                            E_sb[:, tj * NT:(tj + 1) * NT], psl[tj][:], Exp,
                            scale=SCALE, accum_out=acc4[:, tj:tj + 1],
                        )
                    den = stat_pool.tile([P, 1], f32, tag="den")
                    recip = stat_pool.tile([P, 1], f32, tag="recip")
                    diag = stat_pool.tile([P, P], bf16, tag="diag")
                    nc.vector.reduce_sum(den[:], acc4[:], axis=mybir.AxisListType.X)
                    nc.vector.reciprocal(recip[:], den[:])
                    # diag(recip): identity rows scaled per-partition
                    nc.vector.tensor_scalar_mul(diag[:], ident[:], recip[:])
                    # attn^T tile = E-slice.T @ diag  (transpose + normalize),
                    # then the PSUM drain adds intensity^T on DVE.
                    for t in range(TC):
                        pt = tr_ps.tile([P, P], f32, tag="tr")
                        nc.tensor.matmul(
                            pt[:], E_sb[:, t * P:(t + 1) * P], diag[:],
                            start=True, stop=True)
                        nc.vector.tensor_tensor(
                            AT_sb[:, t, si * P:(si + 1) * P],
                            pt[:], IT_sb[:, t * P:(t + 1) * P], add)

            # ---- phase D/E: PV -> out^T, then projection per s-tile ---
            OT_sb = persist.tile([P, DC, SH], bf16, tag="qt_ot")
            with (
                tc.tile_pool(name="wo_pool", bufs=1) as wo_pool,
                tc.tile_pool(name="fin_pool", bufs=3) as fin_pool,
            ):
                WO_sb = wo_pool.tile([P, DC, D], bf16)
                nc.sync.dma_start(WO_sb[:], wo_v)
                for sj in range(SJ):
                    for dvi in range(DC):
                        ps = mm_ps.tile([P, NT], f32, tag="mm")
                        for t in range(TC):
                            nc.tensor.matmul(
                                ps[:],
                                V_sb[:, t, dvi * P:(dvi + 1) * P],
                                AT_sb[:, t, sj * NT:(sj + 1) * NT],
                                start=(t == 0),
                                stop=False,
                            )
                        # bias: bv (x) (1 + rowsum(I))
                        nc.tensor.matmul(
                            ps[:], BV_sb[0:1, dvi * P:(dvi + 1) * P],
                            RS_sb[0:1, sj * NT:(sj + 1) * NT],
                            start=False, stop=True)
                        nc.vector.tensor_copy(
                            out=OT_sb[:, dvi, sj * NT:(sj + 1) * NT], in_=ps[:])

                    for doi in range(DC):
                        ps = mm_ps.tile([P, NT], f32, tag="mm")
                        for dvc in range(DC):
                            nc.tensor.matmul(
                                ps[:],
                                WO_sb[:, dvc, doi * P:(doi + 1) * P],
                                OT_sb[:, dvc, sj * NT:(sj + 1) * NT],
                                start=(dvc == 0), stop=(dvc == DC - 1),
                            )
                        F_sb = fin_pool.tile([P, NT], f32, tag="fin")
                        nc.vector.tensor_scalar_add(
                            F_sb[:], ps[:],
                            BCOL_sb[:, 2 * DC + doi:2 * DC + doi + 1])
                        nc.sync.dma_start(
                            out_v[:, doi, sj * NT:(sj + 1) * NT], F_sb[:])

    nc.compile()
    return nc


def _get_module():
    if "nc" not in _CACHE:
        _CACHE["nc"] = _build_module()
    return _CACHE["nc"]


def _make_in_maps(inputs):
    X = np.asarray(inputs["X"], dtype=np.float32)
    intensity = np.asarray(inputs["intensity"], dtype=np.float32)
    bf = ml_dtypes.bfloat16
    WqT = np.ascontiguousarray(np.asarray(inputs["Wq"], np.float32).T).astype(bf)
    WkT = np.ascontiguousarray(np.asarray(inputs["Wk"], np.float32).T).astype(bf)
    WvT = np.ascontiguousarray(np.asarray(inputs["Wv"], np.float32).T).astype(bf)
    WoT = np.ascontiguousarray(np.asarray(inputs["Wo"], np.float32).T).astype(bf)
    bq, bk, bv, bo = (np.asarray(inputs[k], np.float32).reshape(D)
                      for k in ("bq", "bk", "bv", "bo"))
    BCOL = np.concatenate(
        [b.reshape(DC, P).T for b in (bq, bk, bo)], axis=1
    ).astype(np.float32)  # [128, 24]

    in_maps = []
    for c in range(8):
        b, h = c // 2, c % 2
        XQT = np.ascontiguousarray(X[b, h * SH:(h + 1) * SH, :].T).astype(bf)
        Islc = intensity[b, h * SH:(h + 1) * SH, :]
        # [t, s] -> [si*128+tp, tc*128+sp] so each per-si load is one
        # contiguous row-block (128 descriptors instead of 2048)
        IT = np.ascontiguousarray(
            Islc.T.reshape(TC, P, DC, P).transpose(2, 1, 0, 3).reshape(SH, S)
        ).astype(bf)
        rows = 1.0 + Islc.sum(axis=1, dtype=np.float64).astype(np.float32)
        BROW = np.concatenate([bv, rows]).reshape(1, D + SH)
        in_maps.append({
            "XQT": XQT, "WQT": WqT, "WKT": WkT, "WVT": WvT, "WOT": WoT,
            "BCOL": BCOL, "BROW": BROW, "IT": IT,
        })
    return in_maps


def _gather(results):
    out = np.empty((4, S, D), dtype=np.float32)
    for c in range(8):
        b, h = c // 2, c % 2
        out[b, h * SH:(h + 1) * SH, :] = results[c]["OUTT"].T
    return out


def kernel(**inputs):
    from concourse import bass_utils

    in_maps = _make_in_maps(inputs)
    nc = _get_module()
    res = bass_utils.run_bass_kernel_spmd(nc, in_maps, core_ids=list(range(8)))
    return _gather(res.results)


# revision 29
# speedup vs baseline: 1.2024x; 1.0395x over previous
"""Trainium2 Bass kernel for nn_Attention_54254026883778.

Single-head attention with an additive post-softmax intensity term:
    q/k/v = X @ W{q,k,v}.T + b;  scores = q k^T / sqrt(D)
    attn  = softmax(scores) + intensity;  out = (attn @ v) @ Wo.T + bo

Sharding: 8 cores = 4 batches x 2 sequence halves. Each core computes
K^T and V for its whole batch (duplicated across the pair) and Q/attention
for its own 1024 query rows. No collectives. The host rolls the t-axis per
core so the core's own query rows are always at t-positions 0..SH; K/V and
intensity follow the same rolled order, which leaves attn @ V invariant.

Device dataflow (host pre-transposes X, W and intensity so contraction /
partition dims land where the engines want them):
    Q^T/K^T  [dout | s]  = WxT-chunk.T @ XT        (lhsT=WxT, rhs=XT)
    V        [t | dv]    = XT-chunk.T @ WvT        (lhsT=XT,  rhs=WvT)
    scores   [s | t]     = QT-chunk.T @ KT  -> exp on ACT (no max-subtract:
        |scores| < ~3) with fused row-accumulate -> 1/den on DVE ->
        diag(recip) = ident * recip (DVE) ->
        attn^T tile = E-slice.T @ diag(recip)      (one PE matmul both
        transposes and normalizes) -> DVE copy adds intensity^T (bf16,
        host-transposed) while draining PSUM -> attn^T [t | s]
    out^T    [dv | s]    = V-chunk.T @ attn^T
    final^T  [do | s]    = WoT-chunk.T @ out^T     -> DRAM, host transposes
Biases enter as rank-1 fp32r matmuls accumulated into PSUM:
    q/k/o:  b_row (x) ones_row;   v: bv_row (x) attn-rowsums, where the
    rowsums are 1 + rowsum(intensity) (softmax rows sum to 1), shipped
    from the host inside the bias pack.
"""

import numpy as np
import ml_dtypes

P = 128
D = 1024
S = 2048          # keys per batch (full sequence)
SH = 1024         # query rows owned by each core
DC = D // P       # 8  contraction chunks over model dim
TC = S // P       # 16 t (key) chunks
NT = 512          # matmul moving free dim / psum bank
SJ = SH // NT     # 2  s-tiles of own rows
TJ = S // NT      # 4  t-tiles
SCALE = 1.0 / 32.0  # 1/sqrt(D)

_CACHE = {}


def _build_module():
    import concourse.bass as bass
    import concourse.tile as tile
    import concourse.mybir as mybir
    from concourse import bacc
    from concourse.masks import make_identity

    f32 = mybir.dt.float32
    f32r = mybir.dt.float32r
    bf16 = mybir.dt.bfloat16
    Exp = mybir.ActivationFunctionType.Exp
    add = mybir.AluOpType.add

    nc = bacc.Bacc("TRN2", target_bir_lowering=False, debug=False,
                   num_devices=8)

    XQ_d = nc.dram_tensor("XQT", [D, SH], bf16, kind="ExternalInput")
    WQ_d = nc.dram_tensor("WQT", [D, D], bf16, kind="ExternalInput")
    WK_d = nc.dram_tensor("WKT", [D, D], bf16, kind="ExternalInput")
    WV_d = nc.dram_tensor("WVT", [D, D], bf16, kind="ExternalInput")
    WO_d = nc.dram_tensor("WOT", [D, D], bf16, kind="ExternalInput")
    BCOL_d = nc.dram_tensor("BCOL", [P, 3 * DC], f32, kind="ExternalInput")
    BROW_d = nc.dram_tensor("BROW", [1, D + SH], f32, kind="ExternalInput")
    IT_d = nc.dram_tensor("IT", [SH, S], bf16, kind="ExternalInput")
    OUT_d = nc.dram_tensor("OUTT", [D, SH], f32, kind="ExternalOutput")

    xq_v = XQ_d[:].rearrange("(c p) s -> p c s", p=P)
    wq_v = WQ_d[:].rearrange("(c p) o -> p c o", p=P)
    wk_v = WK_d[:].rearrange("(c p) o -> p c o", p=P)
    wv_v = WV_d[:].rearrange("(c p) o -> p c o", p=P)
    wo_v = WO_d[:].rearrange("(c p) o -> p c o", p=P)
    it_v = IT_d[:].rearrange("(si p) f -> si p f", p=P)  # per-si blocks
    out_v = OUT_d[:].rearrange("(c p) s -> p c s", p=P)

    with tile.TileContext(nc) as tc:
        with (
            tc.tile_pool(name="persist", bufs=1) as persist,
            tc.tile_pool(name="mm_ps", bufs=5, space="PSUM") as mm_ps,
            tc.tile_pool(name="tr_ps", bufs=3, space="PSUM") as tr_ps,
            tc.tile_pool(name="dram", bufs=1, space="DRAM") as dram_pool,
        ):
            GROUPS = [[0, 1], [2, 3], [4, 5], [6, 7]]
            # ---- persistent tiles -------------------------------------
            KT_sb = persist.tile([P, 2, SJ, DC, NT], bf16)  # K^T [d | blk,tt]
            V_sb = persist.tile([P, TC, D], bf16)           # V natural [t | dv]
            QT_sb = persist.tile([P, DC, SH], bf16, tag="qt_ot")   # Q^T [d | s]

            ident = persist.tile([P, P], bf16)
            make_identity(nc, ident)
            # bq|bk|bo as per-partition columns, added during PSUM extract
            BCOL_sb = persist.tile([P, 3 * DC], f32)
            nc.sync.dma_start(BCOL_sb[:], BCOL_d[:])
            # bv and the attn rowsums feed the PV rank-1 bias matmul; fp32r
            # operands must come from a rounding instruction, so stage the
            # DMA through a DVE copy.
            BROW_r = persist.tile([1, D + SH], f32r)
            with tc.tile_pool(name="brow_pool", bufs=1) as brow_pool:
                BROW_ld = brow_pool.tile([1, D + SH], f32)
                nc.sync.dma_start(BROW_ld[:], BROW_d[:])
                nc.vector.tensor_copy(out=BROW_r[:], in_=BROW_ld[:])
            BV_sb = BROW_r[0:1, 0:D]
            RS_sb = BROW_r[0:1, D:D + SH]                   # 1 + rowsum(I)

            # ---- phase A: QKV (local rows) + pair AllGather of K/V ----
            # Each core computes Q^T, and K^T/V for ITS OWN 1024 rows only
            # (from XQT); the partner's half arrives via a 2-rank AllGather
            # through shared DRAM, placed in global t-order (SPMD-static).
            XQ_sb = persist.tile([P, DC, SH], bf16, tag="xq_at")
            KL_sb = persist.tile([P, SJ, DC, NT], bf16, tag="kl_e")
            VL_sb = persist.tile([P, DC, SH], bf16)      # local V [t-own | dv]
            with tc.tile_pool(name="wpool", bufs=2) as wpool:
                WK_sb = wpool.tile([P, DC, D], bf16, tag="w")
                WQ_sb = wpool.tile([P, DC, D], bf16, tag="w")
                # Per-chunk DMAs, K operands first: K-local runs first so its
                # AllGather kicks off as early as possible and hides under
                # the Q and V-local compute.
                for dc in range(DC):
                    nc.sync.dma_start(XQ_sb[:, dc, :], xq_v[:, dc, :])
                    nc.sync.dma_start(WK_sb[:, dc, :], wk_v[:, dc, :])
                for dc in range(DC):
                    nc.sync.dma_start(WQ_sb[:, dc, :], wq_v[:, dc, :])

                # K^T local [dout, t-own], t-tile outer so each half's
                # AllGather kicks off as soon as that half is computed
                for j in range(SJ):
                    for c in range(DC):
                        ps = mm_ps.tile([P, NT], f32, tag="mm", name="ps")
                        for dc in range(DC):
                            nc.tensor.matmul(
                                ps[:],
                                WK_sb[:, dc, c * P:(c + 1) * P],
                                XQ_sb[:, dc, j * NT:(j + 1) * NT],
                                start=(dc == 0), stop=(dc == DC - 1),
                            )
                        nc.vector.tensor_scalar_add(
                            KL_sb[:, j, c, :], ps[:],
                            BCOL_sb[:, DC + c:DC + c + 1])
                    k_in = dram_pool.tile([P, DC, NT], bf16, name="k_in")
                    k_out = dram_pool.tile([2, P, DC, NT], bf16, name="k_out")
                    nc.gpsimd.dma_start(k_in[:], KL_sb[:, j])
                    nc.gpsimd.collective_compute(
                        "AllGather", mybir.AluOpType.bypass,
                        replica_groups=GROUPS,
                        ins=[k_in.opt()], outs=[k_out.opt()])
                    nc.gpsimd.dma_start(KT_sb[:, 0, j], k_out[0])
                    nc.gpsimd.dma_start(KT_sb[:, 1, j], k_out[1])

                # Q^T [dout, s-own]
                for c in range(DC):
                    psl = [mm_ps.tile([P, NT], f32, tag="mm", name="ps")
                           for _ in range(SJ)]
                    for dc in range(DC):
                        for j in range(SJ):
                            nc.tensor.matmul(
                                psl[j][:],
                                WQ_sb[:, dc, c * P:(c + 1) * P],
                                XQ_sb[:, dc, j * NT:(j + 1) * NT],
                                start=(dc == 0), stop=(dc == DC - 1),
                            )
                    for j in range(SJ):
                        nc.vector.tensor_scalar_add(
                            QT_sb[:, c, j * NT:(j + 1) * NT], psl[j][:],
                            BCOL_sb[:, c:c + 1])

                WV_sb = wpool.tile([P, DC, D], bf16, tag="w")
                for dc in range(DC):
                    nc.sync.dma_start(WV_sb[:, dc, :], wv_v[:, dc, :])
                # V local [t-own, dv]; one stationary serves both dv tiles
                for t in range(DC):
                    psl = [mm_ps.tile([P, NT], f32, tag="mm", name="ps")
                           for _ in range(D // NT)]
                    for dc in range(DC):
                        for j in range(D // NT):
                            nc.tensor.matmul(
                                psl[j][:],
                                XQ_sb[:, dc, t * P:(t + 1) * P],
                                WV_sb[:, dc, j * NT:(j + 1) * NT],
                                start=(dc == 0),
                                stop=(dc == DC - 1),
                            )
                    for j in range(D // NT):
                        nc.vector.tensor_copy(
                            out=VL_sb[:, t, j * NT:(j + 1) * NT], in_=psl[j][:])
                v_in = dram_pool.tile([P, DC, SH], bf16, name="v_in")
                v_out = dram_pool.tile([2, P, DC, SH], bf16, name="v_out")
                nc.sync.dma_start(v_in[:], VL_sb[:])
                nc.gpsimd.collective_compute(
                    "AllGather", mybir.AluOpType.bypass,
                    replica_groups=GROUPS,
                    ins=[v_in.opt()], outs=[v_out.opt()])
                nc.sync.dma_start(V_sb[:, 0:DC, :], v_out[0])
                nc.sync.dma_start(V_sb[:, DC:TC, :], v_out[1])
            # ---- phase B/C: scores -> softmax -> +I^T -> attn^T -------
            # Two passes over the key blocks: pass 1 uses only the first
            # gathered K block (ready right after the first AllGather), so
            # the PE never waits for the second; pass 2 finishes the rows
            # and runs the softmax/transpose pipeline.
            AT_sb = persist.tile([P, TC, SH], bf16, tag="xq_at")  # attn^T
            E_sb = persist.tile([P, DC, S], bf16, tag="kl_e")     # exp(scores)
            ACC_sb = persist.tile([P, DC, TJ], f32)
            with (
                tc.tile_pool(name="it_pool", bufs=3) as it_pool,
                tc.tile_pool(name="stat_pool", bufs=2) as stat_pool,
            ):
                for si in range(DC):  # pass 1: K block 0
                    psl = [mm_ps.tile([P, NT], f32, tag="mm", name="ps")
                           for _ in range(SJ)]
                    for dc in range(DC):
                        for u in range(SJ):
                            nc.tensor.matmul(
                                psl[u][:],
                                QT_sb[:, dc, si * P:(si + 1) * P],
                                KT_sb[:, 0, u, dc, :],
                                start=(dc == 0),
                                stop=(dc == DC - 1),
                            )
                    for u in range(SJ):
                        nc.scalar.activation(
                            E_sb[:, si, u * NT:(u + 1) * NT], psl[u][:], Exp,
                            scale=SCALE, accum_out=ACC_sb[:, si, u:u + 1],
                        )
                for si in range(DC):  # pass 2: K block 1 + softmax pipeline
                    IT_sb = it_pool.tile([P, S], bf16, tag="it")
                    nc.sync.dma_start(IT_sb[:], it_v[si])
                    psl = [mm_ps.tile([P, NT], f32, tag="mm", name="ps")
                           for _ in range(SJ)]
                    for dc in range(DC):
                        for u in range(SJ):
                            nc.tensor.matmul(
                                psl[u][:],
                                QT_sb[:, dc, si * P:(si + 1) * P],
                                KT_sb[:, 1, u, dc, :],
                                start=(dc == 0),
                                stop=(dc == DC - 1),
                            )
                    for u in range(SJ):
                        nc.scalar.activation(
                            E_sb[:, si, SH + u * NT:SH + (u + 1) * NT],
                            psl[u][:], Exp,
                            scale=SCALE, accum_out=ACC_sb[:, si, SJ + u:SJ + u + 1],
                        )
                    den = stat_pool.tile([P, 1], f32, tag="den")
                    recip = stat_pool.tile([P, 1], f32, tag="recip")
                    diag = stat_pool.tile([P, P], bf16, tag="diag")
                    nc.vector.reduce_sum(
                        den[:], ACC_sb[:, si, :], axis=mybir.AxisListType.X)
                    nc.vector.reciprocal(recip[:], den[:])
                    # diag(recip): identity rows scaled per-partition
                    nc.vector.tensor_scalar_mul(diag[:], ident[:], recip[:])
                    # attn^T tile = E-slice.T @ diag (transpose + normalize),
                    # then the PSUM drain adds intensity^T on DVE.
                    for t in range(TC):
                        pt = tr_ps.tile([P, P], f32, tag="tr")
                        nc.tensor.matmul(
                            pt[:], E_sb[:, si, t * P:(t + 1) * P], diag[:],
                            start=True, stop=True)
                        nc.vector.tensor_tensor(
                            AT_sb[:, t, si * P:(si + 1) * P],
                            pt[:], IT_sb[:, t * P:(t + 1) * P], add)

            # ---- phase D/E: PV -> out^T, then projection per s-tile ---
            OT_sb = persist.tile([P, DC, SH], bf16, tag="qt_ot")
            with (
                tc.tile_pool(name="wo_pool", bufs=1) as wo_pool,
                tc.tile_pool(name="fin_pool", bufs=3) as fin_pool,
            ):
                WO_sb = wo_pool.tile([P, DC, D], bf16)
                nc.sync.dma_start(WO_sb[:], wo_v)
                for sj in range(SJ):
                    for dvi in range(DC):
                        ps = mm_ps.tile([P, NT], f32, tag="mm")
                        for t in range(TC):
                            nc.tensor.matmul(
                                ps[:],
                                V_sb[:, t, dvi * P:(dvi + 1) * P],
                                AT_sb[:, t, sj * NT:(sj + 1) * NT],
                                start=(t == 0),
                                stop=False,
                            )
                        # bias: bv (x) (1 + rowsum(I))
                        nc.tensor.matmul(
                            ps[:], BV_sb[0:1, dvi * P:(dvi + 1) * P],
                            RS_sb[0:1, sj * NT:(sj + 1) * NT],
                            start=False, stop=True)
                        nc.vector.tensor_copy(
                            out=OT_sb[:, dvi, sj * NT:(sj + 1) * NT], in_=ps[:])

                    for doi in range(DC):
                        ps = mm_ps.tile([P, NT], f32, tag="mm")
                        for dvc in range(DC):
                            nc.tensor.matmul(
                                ps[:],
                                WO_sb[:, dvc, doi * P:(doi + 1) * P],
                                OT_sb[:, dvc, sj * NT:(sj + 1) * NT],
                                start=(dvc == 0), stop=(dvc == DC - 1),
                            )
                        F_sb = fin_pool.tile([P, NT], f32, tag="fin")
                        nc.vector.tensor_scalar_add(
                            F_sb[:], ps[:],
                            BCOL_sb[:, 2 * DC + doi:2 * DC + doi + 1])
                        nc.sync.dma_start(
                            out_v[:, doi, sj * NT:(sj + 1) * NT], F_sb[:])

    nc.compile()
    return nc


def _get_module():
    if "nc" not in _CACHE:
        _CACHE["nc"] = _build_module()
    return _CACHE["nc"]


def _make_in_maps(inputs):
    X = np.asarray(inputs["X"], dtype=np.float32)
    intensity = np.asarray(inputs["intensity"], dtype=np.float32)
    bf = ml_dtypes.bfloat16
    WqT = np.ascontiguousarray(np.asarray(inputs["Wq"], np.float32).T).astype(bf)
    WkT = np.ascontiguousarray(np.asarray(inputs["Wk"], np.float32).T).astype(bf)
    WvT = np.ascontiguousarray(np.asarray(inputs["Wv"], np.float32).T).astype(bf)
    WoT = np.ascontiguousarray(np.asarray(inputs["Wo"], np.float32).T).astype(bf)
    bq, bk, bv, bo = (np.asarray(inputs[k], np.float32).reshape(D)
                      for k in ("bq", "bk", "bv", "bo"))
    BCOL = np.concatenate(
        [b.reshape(DC, P).T for b in (bq, bk, bo)], axis=1
    ).astype(np.float32)  # [128, 24]

    in_maps = []
    for c in range(8):
        b, h = c // 2, c % 2
        XQT = np.ascontiguousarray(X[b, h * SH:(h + 1) * SH, :].T).astype(bf)
        Islc = intensity[b, h * SH:(h + 1) * SH, :]
        # [t, s] -> [si*128+tp, tc*128+sp] so each per-si load is one
        # contiguous row-block (128 descriptors instead of 2048)
        IT = np.ascontiguousarray(
            Islc.T.reshape(TC, P, DC, P).transpose(2, 1, 0, 3).reshape(SH, S)
        ).astype(bf)
        rows = 1.0 + Islc.sum(axis=1, dtype=np.float64).astype(np.float32)
        BROW = np.concatenate([bv, rows]).reshape(1, D + SH)
        in_maps.append({
            "XQT": XQT, "WQT": WqT, "WKT": WkT, "WVT": WvT, "WOT": WoT,
            "BCOL": BCOL, "BROW": BROW, "IT": IT,
        })
    return in_maps


def _gather(results):
    out = np.empty((4, S, D), dtype=np.float32)
    for c in range(8):
        b, h = c // 2, c % 2
        out[b, h * SH:(h + 1) * SH, :] = results[c]["OUTT"].T
    return out


def kernel(**inputs):
    from concourse import bass_utils

    in_maps = _make_in_maps(inputs)
    nc = _get_module()
    res = bass_utils.run_bass_kernel_spmd(nc, in_maps, core_ids=list(range(8)))
    return _gather(res.results)
